# revision 1
# baseline (speedup 1.0000x reference)
"""AtomwiseReadout segment-reduce kernel for 8 TRN2 NeuronCores — v2.

reference computation:
    atomwise = f @ w_e + z_bias[z]            # [N, 1]
    e_total  = segment_sum(atomwise, seg)     # [B, 1], 20 atoms per molecule

Strategy v2 (molecule-per-partition, contiguous DMA):
  - atoms sharded contiguously at molecule boundaries across 8 cores; each
    core runs 98 "supers" of 2560 atoms (= 128 molecules).  Within a super,
    partition p holds atoms [20p, 20p+20) = exactly molecule p, so every
    partition's f data is ONE contiguous 10 KB chunk in HBM (vs 20 strided
    512 B chunks in v1) -> far better DMA descriptor shape.
  - supers are batched B=4 per DMA (5.2 MB per cast-DMA, 512 descriptors
    of 10 KB each).
  - per batch, on-device:
      F tile [128, B*20*128]  one f32->bf16 cast-DMA (SWDGE)
      S[p, j*128+d] = sum_a F[p, j, a, d]   (20 accumulating identity
                                             matmuls of N=B*128)
      z_rep[p, v*20+a] = z[., a]            (Scalar broadcast so the compare
                                             is packed bf16 on the DVE)
      oh[p, j, v*20+a] = (z == v)           (B DVE is_equal vs iota)
      H[p, j, v] = sum_a oh[p, j, v, a]     (DVE bf16 reduce over innermost
                                             a; exact, counts <= 20.  The PE
                                             only ever does the S matmuls so
                                             HAM throttling cannot stall the
                                             DMA stream; z_rep/eq/H all
                                             depend only on z and run
                                             EQ_LEAD batches ahead)
      e[p, j] = reduce([S * w | H * zb])    (2 DVE mults + 1 DVE reduce)
    e[p, j] is molecule (batch*4+j)*128 + p; accumulated in an SBUF result
    tile, one DMA out at the end.
"""

import numpy as np
import ml_dtypes

import concourse.bass as bass
import concourse.bacc as bacc
import concourse.mybir as mybir
import concourse.tile as tile
from concourse.bass_utils import run_bass_kernel_spmd


def _ensure_ntff_hook():
    """Restore the NTFF profile hook if the image's antenv lacks axon_hooks.

    trn_boot.boot() registers this hook at interpreter start, but degrades
    silently when ``antenv.axon_hooks`` is missing — and bass_utils then
    crashes on the import when trace=True. Recreate the module with the
    same hook boot() would have installed. No-op when the real module
    exists.
    """
    try:
        import antenv.axon_hooks  # noqa: F401

        return
    except ImportError:
        pass
    try:
        import sys
        import types

        from trn_agent_boot.trn_boot import _ntff_profile_via_ctypes

        hook = _ntff_profile_via_ctypes("/opt/axon/libaxon_pjrt.so")
        mod = types.ModuleType("antenv.axon_hooks")
        mod.get_axon_ntff_profile_hook = lambda: hook
        mod.set_axon_ntff_profile_hook = lambda h: None
        sys.modules["antenv.axon_hooks"] = mod
    except Exception:
        pass


_ensure_ntff_hook()

# problem constants (hardcoded per spec)
N_ATOMS = 2_000_000
N_MOL = 100_000
APM = 20          # atoms per molecule
D = 128           # feature dim
V = 86            # z vocabulary (0..85)
N_CORES = 8

# tiling
P = 128                       # partitions
SUP_ATOMS = P * APM           # 2560 atoms per super (1 molecule/partition)
SUP_MOLS = P                  # 128 molecules per super
N_SUP = 98                    # supers per core
B = 4                         # supers per DMA/PSUM batch
SHARD_ATOMS = N_SUP * SUP_ATOMS   # 250880
SHARD_MOLS = SHARD_ATOMS // APM   # 12544
MOLS_PER_CORE = N_MOL // N_CORES  # 12500

F32 = mybir.dt.float32
BF16 = mybir.dt.bfloat16

TRACE = False  # test harness can flip this to get a profile
# If True, pre-broadcast z on the Scalar engine so the DVE compare is fully
# packed bf16 (2x mode); if False, the DVE compare reads z via a stride-0
# broadcast AP directly (1x, but frees the Scalar engine).
USE_SCALAR_ZREP = True


def _batches(n_sup):
    """Batch the supers: B-sized batches, tapering at the end ([...4, 2, 2, 1, 1])
    so the post-DMA compute drain after the last f byte is short."""
    taper = [2, 2, 1, 1]
    out = []
    s = 0
    while s < n_sup:
        left = n_sup - s
        if left > sum(taper):
            out.append((s, B))
            s += B
        else:
            for nb in taper:
                if s < n_sup:
                    nb = min(nb, n_sup - s)
                    out.append((s, nb))
                    s += nb
    return out


# how many batches ahead of the flush the one-hot compares are emitted; keeps
# the DVE FIFO from serializing eq(b+1) behind the PSUM-dependent flush(b)
EQ_LEAD = 2


def build(nc, n_sup=N_SUP):
    shard_atoms = n_sup * SUP_ATOMS
    batches = _batches(n_sup)

    f = nc.dram_tensor("f", [shard_atoms, D], F32, kind="ExternalInput")
    zc = nc.dram_tensor("z_cols", [P, n_sup * APM], BF16, kind="ExternalInput")
    ident = nc.dram_tensor("ident", [P, P], BF16, kind="ExternalInput")
    iota = nc.dram_tensor("iota", [P, APM * V], BF16, kind="ExternalInput")
    w = nc.dram_tensor("w_rep", [P, B * D], F32, kind="ExternalInput")
    zb = nc.dram_tensor("zb_rep", [P, B * V], BF16, kind="ExternalInput")
    out = nc.dram_tensor("out", [P, n_sup], F32, kind="ExternalOutput")

    # atom row = n*2560 + p*20 + a  ->  [p, n, (a d)]: per (p, n) the HBM
    # data is one contiguous 20*128*4B = 10 KB run.
    fv = f.ap().rearrange("(n p a) d -> p n (a d)", p=P, a=APM)

    with tile.TileContext(nc) as tc:
        with (
            tc.tile_pool(name="const", bufs=1) as cpool,
            tc.tile_pool(name="fpool", bufs=4) as fpool,
            tc.tile_pool(name="ohpool", bufs=2 + EQ_LEAD) as ohpool,
            tc.tile_pool(name="work", bufs=2) as pool,
            tc.tile_pool(name="zrep", bufs=8) as zrpool,
            tc.tile_pool(name="hist", bufs=4) as hhpool,
            tc.tile_pool(name="psum_s", bufs=4, space="PSUM") as ppool_s,
        ):
            # first f DMA goes first so the SWDGE stream starts immediately;
            # consts ride the HWDGE (sync) queue in parallel
            f_tiles = {}

            def emit_fdma(bi):
                sup0, nb = batches[bi]
                f_sb = fpool.tile([P, B * SUP_ATOMS], BF16, tag="f")
                nc.gpsimd.dma_start(
                    out=f_sb[:, : nb * SUP_ATOMS],
                    in_=fv[:, sup0 : sup0 + nb, :],
                )
                f_tiles[bi] = f_sb

            emit_fdma(0)

            zc_sb = cpool.tile([P, n_sup * APM], BF16)
            nc.sync.dma_start(out=zc_sb[:], in_=zc.ap())
            ident_sb = cpool.tile([P, P], BF16)
            nc.sync.dma_start(out=ident_sb[:], in_=ident.ap())
            iota_sb = cpool.tile([P, APM * V], BF16)
            nc.sync.dma_start(out=iota_sb[:], in_=iota.ap())
            w_sb = cpool.tile([P, B * D], F32)
            nc.sync.dma_start(out=w_sb[:], in_=w.ap())
            zb_sb = cpool.tile([P, B * V], BF16)
            nc.sync.dma_start(out=zb_sb[:], in_=zb.ap())

            # DVE-local copies of every tile a DVE instruction reads, so
            # those instructions never need DMA-semaphore waits.
            zc2 = cpool.tile([P, n_sup * APM], BF16)
            nc.vector.tensor_copy(out=zc2[:], in_=zc_sb[:])
            iota2 = cpool.tile([P, APM * V], BF16)
            nc.vector.tensor_copy(out=iota2[:], in_=iota_sb[:])
            w2 = cpool.tile([P, B * D], F32)
            nc.vector.tensor_copy(out=w2[:], in_=w_sb[:])
            zb2 = cpool.tile([P, B * V], BF16)
            nc.vector.tensor_copy(out=zb2[:], in_=zb_sb[:])

            res = cpool.tile([P, n_sup], F32)
            # super index after the last full-size batch (0 = no split)
            split = max([s0 + nb for s0, nb in batches if nb == B], default=0)

            oh_tiles = {}
            hh_tiles = {}

            def emit_eq(bi):
                """One-hot compares + histogram for batch bi, (j, v, a) layout.

                The Scalar engine pre-broadcasts z across the one-hot width so
                the DVE compare is fully packed bf16 (flat in/out); the DVE
                then reduces the innermost a axis into bf16 molecule counts
                (exact: counts <= 20). Everything here depends only on z, so
                it runs EQ_LEAD batches ahead of the f stream.
                """
                sup0, nb = batches[bi]
                oh = ohpool.tile([P, B * APM * V], BF16, tag="oh")
                for j in range(nb):
                    s_idx = sup0 + j
                    z_ap = (
                        zc2[:, s_idx * APM : (s_idx + 1) * APM]
                        .unsqueeze(1)
                        .to_broadcast([P, V, APM])
                    )
                    if USE_SCALAR_ZREP:
                        z_rep = zrpool.tile([P, APM * V], BF16, tag="zr")
                        nc.scalar.copy(out=z_rep[:], in_=z_ap)
                        z_in = z_rep[:]
                    else:
                        z_in = z_ap
                    nc.vector.tensor_tensor(
                        out=oh[:, j * APM * V : (j + 1) * APM * V],
                        in0=iota2[:],
                        in1=z_in,
                        op=mybir.AluOpType.is_equal,
                    )
                hh = hhpool.tile([P, B * V], BF16, tag="hh")
                ohg = oh[:].rearrange("p (jv a) -> p jv a", a=APM)
                with nc.allow_low_precision(
                    reason="histogram counts <= 20 are exact in bf16"
                ):
                    nc.vector.tensor_reduce(
                        out=hh[:, : nb * V],
                        in_=ohg[:, : nb * V, :],
                        axis=mybir.AxisListType.X,
                        op=mybir.AluOpType.add,
                    )
                oh_tiles[bi] = oh
                hh_tiles[bi] = hh

            for bi in range(min(EQ_LEAD, len(batches))):
                emit_eq(bi)

            for bi, (sup0, nb) in enumerate(batches):
                if bi > 0:
                    emit_fdma(bi)
                if bi + EQ_LEAD < len(batches):
                    emit_eq(bi + EQ_LEAD)
                f_sb = f_tiles.pop(bi)
                oh_tiles.pop(bi)
                hh = hh_tiles.pop(bi)
                # [p, (j a d)] -> [p, a, j, d] so each a-slice is a matmul rhs
                fva = f_sb[:].rearrange("p (j a d) -> p a j d", a=APM, d=D)

                s_ps = ppool_s.tile([P, B * D], F32, tag="S")
                for a in range(APM):
                    nc.tensor.matmul(
                        out=s_ps[:, : nb * D],
                        lhsT=ident_sb[:],
                        rhs=fva[:, a : a + 1, :nb, :],
                        start=(a == 0),
                        stop=(a == APM - 1),
                    )

                scr = pool.tile([P, B * (D + V)], F32, tag="scr")
                sv = scr[:].rearrange("p (j c) -> p j c", j=B)
                nc.vector.tensor_tensor(
                    out=sv[:, :nb, :D],
                    in0=s_ps[:, : nb * D].rearrange("p (j d) -> p j d", d=D),
                    in1=w2[:, : nb * D].rearrange("p (j d) -> p j d", d=D),
                    op=mybir.AluOpType.mult,
                )
                nc.vector.tensor_tensor(
                    out=sv[:, :nb, D:],
                    in0=hh[:, : nb * V].rearrange("p (j v) -> p j v", v=V),
                    in1=zb2[:, : nb * V].rearrange("p (j v) -> p j v", v=V),
                    op=mybir.AluOpType.mult,
                )
                nc.vector.tensor_reduce(
                    out=res[:, sup0 : sup0 + nb],
                    in_=sv[:, :nb, :],
                    axis=mybir.AxisListType.X,
                    op=mybir.AluOpType.add,
                )
                # ship the bulk of the result as soon as the big batches are
                # done; only a tiny out-DMA remains after the tapered tail
                if sup0 + nb == split and split > 0:
                    nc.sync.dma_start(
                        out=out.ap()[:, :split], in_=res[:, :split]
                    )

            if split > 0:
                nc.sync.dma_start(out=out.ap()[:, split:], in_=res[:, split:])
            else:
                nc.sync.dma_start(out=out.ap(), in_=res[:])
    nc.compile()
    return nc


def _prep_core_inputs(f, z, w_e, z_bias, start, n_sup=N_SUP):
    """Per-core input map. f/z are the full arrays; start = first atom row."""
    shard_atoms = n_sup * SUP_ATOMS
    zs = np.asarray(z[start : start + shard_atoms]).astype(np.float32)
    # z_cols[p, n*20+a] = z[start + n*2560 + 20p + a]
    z_cols = np.ascontiguousarray(
        zs.reshape(n_sup, P, APM).transpose(1, 0, 2).reshape(P, n_sup * APM)
    ).astype(ml_dtypes.bfloat16)
    return {
        "f": f[start : start + shard_atoms],
        "z_cols": z_cols,
        "ident": _IDENT,
        "iota": _IOTA,
        "w_rep": np.ascontiguousarray(
            np.broadcast_to(
                np.tile(np.asarray(w_e, np.float32).reshape(D), B), (P, B * D)
            )
        ),
        "zb_rep": np.ascontiguousarray(
            np.broadcast_to(
                np.tile(np.asarray(z_bias, np.float32).reshape(V), B), (P, B * V)
            )
        ).astype(ml_dtypes.bfloat16),
    }


_IDENT = np.eye(P, dtype=np.float32).astype(ml_dtypes.bfloat16)
# iota in (v, a) layout: iota[p, v*APM + a] = v
_IOTA = np.ascontiguousarray(
    np.broadcast_to(np.repeat(np.arange(V, dtype=np.float32), APM), (P, APM * V))
).astype(ml_dtypes.bfloat16)

_NC_CACHE = {}
_LAST_RESULTS = None  # BassKernelResults of the most recent run (for profiling)


def kernel(z, f, num_atoms, w_e, z_bias):
    global _LAST_RESULTS
    z = np.asarray(z)
    f = np.ascontiguousarray(np.asarray(f, dtype=np.float32))
    w_e = np.asarray(w_e, dtype=np.float32)
    z_bias = np.asarray(z_bias, dtype=np.float32)
    assert f.shape == (N_ATOMS, D)

    key = ("v2", N_SUP, B, USE_SCALAR_ZREP)
    if key not in _NC_CACHE:
        _NC_CACHE[key] = build(bacc.Bacc(), N_SUP)
    nc = _NC_CACHE[key]

    # core i handles molecules [i*12500, (i+1)*12500); its shard starts at
    # atom i*250000 except the last core, whose shard is right-aligned so
    # no padding is ever needed.
    starts = [i * MOLS_PER_CORE * APM for i in range(N_CORES - 1)]
    starts.append(N_ATOMS - SHARD_ATOMS)
    in_maps = [_prep_core_inputs(f, z, w_e, z_bias, s) for s in starts]

    res = run_bass_kernel_spmd(nc, in_maps, core_ids=list(range(N_CORES)), trace=TRACE)
    _LAST_RESULTS = res

    out = np.empty((N_MOL, 1), np.float32)
    for i in range(N_CORES):
        # device layout: out[p, n] = e of molecule n*128 + p (within shard)
        arr = np.asarray(res.results[i]["out"])  # [P, n_sup]
        e = arr.T.reshape(N_SUP * SUP_MOLS)
        first_mol = starts[i] // APM
        lo = i * MOLS_PER_CORE
        out[lo : lo + MOLS_PER_CORE, 0] = e[lo - first_mol : lo - first_mol + MOLS_PER_CORE]
    return out



# revision 3
# speedup vs baseline: 1.1929x; 1.1929x over previous
"""AtomwiseReadout segment-reduce kernel for 8 TRN2 NeuronCores — v3.

reference computation:
    atomwise = f @ w_e + z_bias[z]            # [N, 1]
    e_total  = segment_sum(atomwise, seg)     # [B, 1], 20 atoms per molecule

Strategy v3 (molecule-per-partition, contiguous DMA, LUT-by-sweep bias):
  - atoms sharded contiguously at molecule boundaries across 8 cores; each
    core runs 98 "supers" of 2560 atoms (= 128 molecules).  Within a super,
    partition p holds atoms [20p, 20p+20) = exactly molecule p, so every
    partition's f data is ONE contiguous 10 KB chunk in HBM -> fat DMA
    descriptors at line rate.
  - f streams in B=2 supers per cast-DMA (2.6 MB, f32->bf16 SWDGE).  B=2
    keeps the PE's inter-batch idle gap (~2.6us) under the ~5.2us HAM
    re-throttle window so the 20 accumulating identity matmuls per batch
    (PSUM a-sum) run at the warm 2.4 GHz clock.  v2 used B=4, whose 7.7us
    gaps re-throttled the PE every batch (measured 480ns/matmul avg).
  - z_bias[z] is evaluated by an 85-pass value sweep on the DVE instead of
    v2's one-hot/histogram (which kept DVE 85% + Scalar 42% busy and
    starved the DMA ring):
        for v in 1..85:  m = (z == v) * zb[v]        (tensor_scalar, 4x bf16)
                         acc ^= acc + m              (tensor_tensor, 2x bf16)
    Each z matches exactly one v, so every acc element is assigned once and
    otherwise accumulates zeros -> the sweep is EXACT in bf16.  Two
    accumulators (even/odd v) break the output-dependence chain so the DVE
    pipeline never stalls on its own output hazard.  Total DVE load drops
    to ~45%, Scalar to 0%.
  - per batch flush: S*w mult + reduce -> res[p, super].  At super 96 the
    bias total (one reduce over acc0+acc1) is folded in and the bulk of the
    result DMAs out; a 2-super taper keeps the post-stream drain short.
"""

import numpy as np
import ml_dtypes

import concourse.bass as bass
import concourse.bacc as bacc
import concourse.mybir as mybir
import concourse.tile as tile
from concourse.bass_utils import run_bass_kernel_spmd


def _ensure_ntff_hook():
    """Restore the NTFF profile hook if the image's antenv lacks axon_hooks.

    trn_boot.boot() registers this hook at interpreter start, but degrades
    silently when ``antenv.axon_hooks`` is missing — and bass_utils then
    crashes on the import when trace=True. Recreate the module with the
    same hook boot() would have installed. No-op when the real module
    exists.
    """
    try:
        import antenv.axon_hooks  # noqa: F401

        return
    except ImportError:
        pass
    try:
        import sys
        import types

        from trn_agent_boot.trn_boot import _ntff_profile_via_ctypes

        hook = _ntff_profile_via_ctypes("/opt/axon/libaxon_pjrt.so")
        mod = types.ModuleType("antenv.axon_hooks")
        mod.get_axon_ntff_profile_hook = lambda: hook
        mod.set_axon_ntff_profile_hook = lambda h: None
        sys.modules["antenv.axon_hooks"] = mod
    except Exception:
        pass


_ensure_ntff_hook()

# problem constants (hardcoded per spec)
N_ATOMS = 2_000_000
N_MOL = 100_000
APM = 20          # atoms per molecule
D = 128           # feature dim
V = 86            # z vocabulary (0..85); z values are in [1, 85]
N_CORES = 8

# tiling
P = 128                       # partitions
SUP_ATOMS = P * APM           # 2560 atoms per super (1 molecule/partition)
SUP_MOLS = P                  # 128 molecules per super
N_SUP = 98                    # supers per core
B = 2                         # supers per DMA/PSUM batch
SHARD_ATOMS = N_SUP * SUP_ATOMS   # 250880
SHARD_MOLS = SHARD_ATOMS // APM   # 12544
MOLS_PER_CORE = N_MOL // N_CORES  # 12500

F32 = mybir.dt.float32
BF16 = mybir.dt.bfloat16

TRACE = False  # test harness can flip this to get a profile
FBUFS = 8      # f-tile double buffering depth (10 KB/partition each)
PRE_PASSES = 8   # bias sweep passes emitted before the batch loop
PASSES_PER_BATCH = 2


def _batches(n_sup):
    """B-sized batches with a [1, 1] taper so the drain after the last f
    byte is one tiny super."""
    out = [(s, B) for s in range(0, n_sup - 2, B)]
    out += [(n_sup - 2, 1), (n_sup - 1, 1)]
    return out


def build(nc, n_sup=N_SUP):
    shard_atoms = n_sup * SUP_ATOMS
    batches = _batches(n_sup)
    za = n_sup * APM  # z elements per partition

    f = nc.dram_tensor("f", [shard_atoms, D], F32, kind="ExternalInput")
    zc = nc.dram_tensor("z_cols", [P, za], BF16, kind="ExternalInput")
    ident = nc.dram_tensor("ident", [P, P], BF16, kind="ExternalInput")
    w = nc.dram_tensor("w_rep", [P, B * D], F32, kind="ExternalInput")
    zbb = nc.dram_tensor("zb_bcast", [P, V], F32, kind="ExternalInput")
    out = nc.dram_tensor("out", [P, n_sup], F32, kind="ExternalOutput")

    # atom row = n*2560 + p*20 + a  ->  [p, n, (a d)]: per (p, n) the HBM
    # data is one contiguous 20*128*4B = 10 KB run.
    fv = f.ap().rearrange("(n p a) d -> p n (a d)", p=P, a=APM)

    with tile.TileContext(nc) as tc:
        with (
            tc.tile_pool(name="const", bufs=1) as cpool,
            tc.tile_pool(name="fpool", bufs=FBUFS) as fpool,
            tc.tile_pool(name="work", bufs=2) as pool,
            tc.tile_pool(name="psum_s", bufs=4, space="PSUM") as ppool_s,
        ):
            # first f DMA goes first so the SWDGE stream starts immediately;
            # consts ride the HWDGE (sync) queue in parallel
            f_tiles = {}

            def emit_fdma(bi):
                sup0, nb = batches[bi]
                f_sb = fpool.tile([P, B * SUP_ATOMS], BF16, tag="f")
                nc.gpsimd.dma_start(
                    out=f_sb[:, : nb * SUP_ATOMS],
                    in_=fv[:, sup0 : sup0 + nb, :],
                )
                f_tiles[bi] = f_sb

            emit_fdma(0)

            zc_sb = cpool.tile([P, za], BF16)
            nc.sync.dma_start(out=zc_sb[:], in_=zc.ap())
            ident_sb = cpool.tile([P, P], BF16)
            nc.sync.dma_start(out=ident_sb[:], in_=ident.ap())
            w_sb = cpool.tile([P, B * D], F32)
            nc.sync.dma_start(out=w_sb[:], in_=w.ap())
            zbb_sb = cpool.tile([P, V], F32)
            nc.sync.dma_start(out=zbb_sb[:], in_=zbb.ap())

            # DVE-local copies so the sweep/flush never re-wait DMA sems
            zc2 = cpool.tile([P, za], BF16)
            nc.vector.tensor_copy(out=zc2[:], in_=zc_sb[:])
            w2 = cpool.tile([P, B * D], F32)
            nc.vector.tensor_copy(out=w2[:], in_=w_sb[:])
            zbb2 = cpool.tile([P, V], F32)
            nc.vector.tensor_copy(out=zbb2[:], in_=zbb_sb[:])

            res = cpool.tile([P, n_sup], F32)
            res2 = cpool.tile([P, n_sup], F32)
            bias_tot = cpool.tile([P, n_sup], F32)
            acc = [
                cpool.tile([P, za], BF16, name=f"acc{i}") for i in range(2)
            ]
            mt = [cpool.tile([P, za], BF16, name=f"mt{i}") for i in range(2)]

            # ---- bias value sweep: acc[v%2] += (z == v) * zb[v] ----
            vs = list(range(1, V))  # z values are 1..85
            pass_state = {"k": 0}

            def emit_pass():
                k = pass_state["k"]
                if k >= len(vs):
                    return
                pass_state["k"] = k + 1
                v = vs[k]
                a = acc[k % 2]
                with nc.allow_low_precision(
                    reason="each z matches exactly one v; all other adds are +0"
                ):
                    if k < 2:
                        # first write of this accumulator
                        nc.vector.tensor_scalar(
                            out=a[:], in0=zc2[:],
                            scalar1=float(v), scalar2=zbb2[:, v : v + 1],
                            op0=mybir.AluOpType.is_equal,
                            op1=mybir.AluOpType.mult,
                        )
                    else:
                        m = mt[k % 2]
                        nc.vector.tensor_scalar(
                            out=m[:], in0=zc2[:],
                            scalar1=float(v), scalar2=zbb2[:, v : v + 1],
                            op0=mybir.AluOpType.is_equal,
                            op1=mybir.AluOpType.mult,
                        )
                        nc.vector.tensor_tensor(
                            out=a[:], in0=a[:], in1=m[:],
                            op=mybir.AluOpType.add,
                        )

            for _ in range(PRE_PASSES):
                emit_pass()

            # super index after the last full-size batch
            split = n_sup - 2

            for bi, (sup0, nb) in enumerate(batches):
                if bi > 0:
                    emit_fdma(bi)
                f_sb = f_tiles.pop(bi)
                # [p, (j a d)] -> [p, a, j, d] so each a-slice is a matmul rhs
                fva = f_sb[:].rearrange("p (j a d) -> p a j d", a=APM, d=D)

                s_ps = ppool_s.tile([P, B * D], F32, tag="S")
                for a in range(APM):
                    nc.tensor.matmul(
                        out=s_ps[:, : nb * D],
                        lhsT=ident_sb[:],
                        rhs=fva[:, a : a + 1, :nb, :],
                        start=(a == 0),
                        stop=(a == APM - 1),
                    )

                scr = pool.tile([P, B * D], F32, tag="scr")
                nc.vector.tensor_tensor(
                    out=scr[:, : nb * D],
                    in0=s_ps[:, : nb * D],
                    in1=w2[:, : nb * D],
                    op=mybir.AluOpType.mult,
                )
                nc.vector.tensor_reduce(
                    out=res[:, sup0 : sup0 + nb],
                    in_=scr[:].rearrange("p (j d) -> p j d", d=D)[:, :nb, :],
                    axis=mybir.AxisListType.X,
                    op=mybir.AluOpType.add,
                )
                for _ in range(PASSES_PER_BATCH):
                    emit_pass()

                if sup0 + nb == split:
                    # all sweep passes are done by now (8 + 2/batch >= 85
                    # by iteration ~39 << 48); fold the bias in and ship
                    # the bulk of the result while the taper streams.
                    assert pass_state["k"] == len(vs)
                    asum = mt[0]  # sweep scratch is dead now; reuse
                    nc.vector.tensor_tensor(
                        out=asum[:], in0=acc[0][:], in1=acc[1][:],
                        op=mybir.AluOpType.add,
                    )
                    nc.vector.tensor_reduce(
                        out=bias_tot[:],
                        in_=asum[:].rearrange("p (n a) -> p n a", a=APM),
                        axis=mybir.AxisListType.X,
                        op=mybir.AluOpType.add,
                    )
                    nc.vector.tensor_tensor(
                        out=res2[:, :split], in0=res[:, :split],
                        in1=bias_tot[:, :split], op=mybir.AluOpType.add,
                    )
                    nc.sync.dma_start(
                        out=out.ap()[:, :split], in_=res2[:, :split]
                    )

            nc.vector.tensor_tensor(
                out=res2[:, split:], in0=res[:, split:],
                in1=bias_tot[:, split:], op=mybir.AluOpType.add,
            )
            nc.sync.dma_start(out=out.ap()[:, split:], in_=res2[:, split:])
    nc.compile()
    return nc


def _prep_core_inputs(f, z, w_e, z_bias, start, n_sup=N_SUP):
    """Per-core input map. f/z are the full arrays; start = first atom row."""
    shard_atoms = n_sup * SUP_ATOMS
    zs = np.asarray(z[start : start + shard_atoms]).astype(np.float32)
    # z_cols[p, n*20+a] = z[start + n*2560 + 20p + a]
    z_cols = np.ascontiguousarray(
        zs.reshape(n_sup, P, APM).transpose(1, 0, 2).reshape(P, n_sup * APM)
    ).astype(ml_dtypes.bfloat16)
    return {
        "f": f[start : start + shard_atoms],
        "z_cols": z_cols,
        "ident": _IDENT,
        "w_rep": np.ascontiguousarray(
            np.broadcast_to(
                np.tile(np.asarray(w_e, np.float32).reshape(D), B), (P, B * D)
            )
        ),
        "zb_bcast": np.ascontiguousarray(
            np.broadcast_to(
                np.asarray(z_bias, np.float32).reshape(1, V), (P, V)
            )
        ),
    }


_IDENT = np.eye(P, dtype=np.float32).astype(ml_dtypes.bfloat16)

_NC_CACHE = {}
_LAST_RESULTS = None  # BassKernelResults of the most recent run (for profiling)


def kernel(z, f, num_atoms, w_e, z_bias):
    global _LAST_RESULTS
    z = np.asarray(z)
    f = np.ascontiguousarray(np.asarray(f, dtype=np.float32))
    w_e = np.asarray(w_e, dtype=np.float32)
    z_bias = np.asarray(z_bias, dtype=np.float32)
    assert f.shape == (N_ATOMS, D)

    key = ("v3", N_SUP, B, FBUFS)
    if key not in _NC_CACHE:
        _NC_CACHE[key] = build(bacc.Bacc(), N_SUP)
    nc = _NC_CACHE[key]

    # core i handles molecules [i*12500, (i+1)*12500); its shard starts at
    # atom i*250000 except the last core, whose shard is right-aligned so
    # no padding is ever needed.
    starts = [i * MOLS_PER_CORE * APM for i in range(N_CORES - 1)]
    starts.append(N_ATOMS - SHARD_ATOMS)
    in_maps = [_prep_core_inputs(f, z, w_e, z_bias, s) for s in starts]

    res = run_bass_kernel_spmd(nc, in_maps, core_ids=list(range(N_CORES)), trace=TRACE)
    _LAST_RESULTS = res

    out = np.empty((N_MOL, 1), np.float32)
    for i in range(N_CORES):
        # device layout: out[p, n] = e of molecule n*128 + p (within shard)
        arr = np.asarray(res.results[i]["out"])  # [P, n_sup]
        e = arr.T.reshape(N_SUP * SUP_MOLS)
        first_mol = starts[i] // APM
        lo = i * MOLS_PER_CORE
        out[lo : lo + MOLS_PER_CORE, 0] = e[lo - first_mol : lo - first_mol + MOLS_PER_CORE]
    return out
